# revision 1
# baseline (speedup 1.0000x reference)
"""GCNConv (graph message passing) on 8 Trainium2 NeuronCores — Bass/Tile.

out = a + (a @ Wres + bres),  a = relu(segment_sum(edge_val * (xW+b)[edge_col],
edge_row)),  computed via the identity  agg_lin = (A@x) @ W + deg x b  so the
sparse part runs on raw x.

Sharding: nodes (segment-sum destinations) are partitioned across the 8 cores
(12500 nodes each); x and the small dense weights are replicated; each core
processes exactly the edges whose destination lands in its shard (host-side
routing — the "route messages for cross-partition edges" step of the hint).

Per-core device algorithm (fully transposed, features on partitions):
  Phase 1: for each 128-destination block, gather the needed source rows with
  dma_gather (int16 indices into <=25000-row chunks of x) and accumulate
  psum[f, d] += xg_tile.T @ S over the block's 128-edge tiles, where
  S[e, d] = onehot(dest_in_block(e)) * edge_val(e) is built on the vector
  engine from an iota constant with a single two-op tensor_scalar
  (is_equal then mult).  PSUM accumulates across all source chunks of a
  block, then flushes to an SBUF aggT [128 features, 12544 dests].
  Phase 2 (slabs of 4 blocks): psA = W.T @ aggT_slab + b x deg (rank-1
  matmul), aT = relu(psA); psB = Wres.T @ aT + bres x 1; outT = psB + aT,
  stored transposed [128, 12544] per core; the host transposes + concatenates.
"""
import math
import numpy as np

import concourse.tile as tile
from concourse import bacc, mybir
from concourse.bass_utils import run_bass_kernel_spmd

F32 = mybir.dt.float32
I16 = mybir.dt.int16
AL = mybir.AluOpType
D = 128
P = 128
N_CORES = 8
CH = 25000        # x chunk rows (int16 gather indices => <= 32767)
SBW = 4           # destination blocks per superblock (one gather per chunk)


def _build(n_src, n_blocks, tpb, repeat=1):
    nsh_pad = n_blocks * P
    Q = math.ceil(n_src / CH)
    G = Q * n_blocks * tpb
    IC = G * 8
    sbs = [list(range(s, min(s + SBW, n_blocks))) for s in range(0, n_blocks, SBW)]

    nc = bacc.Bacc("TRN2", target_bir_lowering=False, debug=False)
    x = nc.dram_tensor("x", [n_src, D], F32, kind="ExternalInput")
    W = nc.dram_tensor("W", [D, D], F32, kind="ExternalInput")
    Wres = nc.dram_tensor("Wres", [D, D], F32, kind="ExternalInput")
    bvec = nc.dram_tensor("bvec", [1, D], F32, kind="ExternalInput")
    bres = nc.dram_tensor("bres", [1, D], F32, kind="ExternalInput")
    iotaf = nc.dram_tensor("iotaf", [P, P], F32, kind="ExternalInput")
    idx = nc.dram_tensor("idx", [P, IC], I16, kind="ExternalInput")
    darr = nc.dram_tensor("darr", [P, G], F32, kind="ExternalInput")
    varr = nc.dram_tensor("varr", [P, G], F32, kind="ExternalInput")
    deg = nc.dram_tensor("deg", [1, nsh_pad], F32, kind="ExternalInput")
    outT = nc.dram_tensor("outT", [D, nsh_pad], F32, kind="ExternalOutput")

    with tile.TileContext(nc) as tc:
        with tc.tile_pool(name="const", bufs=1) as cp:
            W_sb = cp.tile([D, D], F32)
            nc.sync.dma_start(W_sb[:], W.ap())
            Wres_sb = cp.tile([D, D], F32)
            nc.sync.dma_start(Wres_sb[:], Wres.ap())
            b_sb = cp.tile([1, D], F32)
            nc.sync.dma_start(b_sb[:], bvec.ap())
            bres_sb = cp.tile([1, D], F32)
            nc.sync.dma_start(bres_sb[:], bres.ap())
            deg_sb = cp.tile([1, nsh_pad], F32)
            nc.sync.dma_start(deg_sb[:], deg.ap())
            iota_f = cp.tile([P, P], F32)
            nc.sync.dma_start(iota_f[:], iotaf.ap())
            idx_sb = cp.tile([P, IC], I16)
            nc.sync.dma_start(idx_sb[:], idx.ap())
            d_sb = cp.tile([P, G], F32)
            nc.sync.dma_start(d_sb[:], darr.ap())
            v_sb = cp.tile([P, G], F32)
            nc.sync.dma_start(v_sb[:], varr.ap())
            ones_row = cp.tile([1, 512], F32)
            nc.vector.memset(ones_row[:], 1.0)
            aggT = cp.tile([D, nsh_pad], F32)

            for _rep in range(repeat):
                # ---- Phase 1: gather + one-hot-matmul segment sum ----
                with (
                    tc.tile_pool(name="xg", bufs=Q + 1) as xg_pool,
                    tc.tile_pool(name="s", bufs=6) as s_pool,
                    tc.tile_pool(name="ps1", bufs=4, space="PSUM") as ps1,
                ):
                    gt = 0
                    cbase = 0
                    for sb in sbs:
                        nb = len(sb)
                        nidx = nb * tpb * P
                        xgs = []
                        for q in range(Q):
                            xg = xg_pool.tile([P, nb * tpb * P], F32, tag="xg",
                                              name=f"xg{q}")
                            nc.gpsimd.dma_gather(
                                xg[:].rearrange("p (t f) -> p t f", f=P),
                                x.ap()[q * CH: min(n_src, (q + 1) * CH), :],
                                idx_sb[:, cbase: cbase + nidx // 16],
                                nidx, nidx, D,
                                single_packet=(nidx <= 1024),
                            )
                            cbase += nidx // 16
                            xgs.append(xg)
                        pss = [ps1.tile([P, P], F32, tag="ps", name=f"ps{j}")
                               for j in range(nb)]
                        for q in range(Q):
                            for j in range(nb):
                                for t in range(tpb):
                                    S = s_pool.tile([P, P], F32, name="S")
                                    nc.vector.tensor_scalar(
                                        S[:], iota_f[:],
                                        d_sb[:, gt:gt + 1], v_sb[:, gt:gt + 1],
                                        op0=AL.is_equal, op1=AL.mult,
                                    )
                                    e0 = (j * tpb + t) * P
                                    nc.tensor.matmul(
                                        out=pss[j][:],
                                        lhsT=xgs[q][:, e0:e0 + P],
                                        rhs=S[:],
                                        start=(q == 0 and t == 0),
                                        stop=(q == Q - 1 and t == tpb - 1),
                                    )
                                    gt += 1
                        for j, k in enumerate(sb):
                            nc.vector.tensor_copy(aggT[:, k * P:(k + 1) * P],
                                                  pss[j][:])

                # ---- Phase 2: dense head ----
                SLAB = 4 * P
                with (
                    tc.tile_pool(name="a", bufs=2) as a_pool,
                    tc.tile_pool(name="o", bufs=2) as o_pool,
                    tc.tile_pool(name="psA", bufs=2, space="PSUM") as psA_pool,
                    tc.tile_pool(name="psB", bufs=2, space="PSUM") as psB_pool,
                ):
                    for s0 in range(0, nsh_pad, SLAB):
                        w = min(SLAB, nsh_pad - s0)
                        psA = psA_pool.tile([P, SLAB], F32)
                        nc.tensor.matmul(out=psA[:, :w], lhsT=W_sb[:],
                                         rhs=aggT[:, s0:s0 + w],
                                         start=True, stop=False)
                        nc.tensor.matmul(out=psA[:, :w], lhsT=b_sb[:1, :],
                                         rhs=deg_sb[:1, s0:s0 + w],
                                         start=False, stop=True)
                        a_t = a_pool.tile([P, SLAB], F32)
                        nc.scalar.activation(a_t[:, :w], psA[:, :w],
                                             mybir.ActivationFunctionType.Relu)
                        psB = psB_pool.tile([P, SLAB], F32)
                        nc.tensor.matmul(out=psB[:, :w], lhsT=Wres_sb[:],
                                         rhs=a_t[:, :w], start=True, stop=False)
                        nc.tensor.matmul(out=psB[:, :w], lhsT=bres_sb[:1, :],
                                         rhs=ones_row[:1, :w],
                                         start=False, stop=True)
                        o_t = o_pool.tile([P, SLAB], F32)
                        nc.vector.tensor_tensor(o_t[:, :w], psB[:, :w],
                                                a_t[:, :w], op=AL.add)
                        nc.sync.dma_start(outT.ap()[:, s0:s0 + w], o_t[:, :w])

    nc.compile()
    return nc


def _prep(x, W, b, Wres, bres, edge_val, edge_row, edge_col):
    x = np.ascontiguousarray(np.asarray(x, np.float32))
    W = np.ascontiguousarray(np.asarray(W, np.float32))
    Wres = np.ascontiguousarray(np.asarray(Wres, np.float32))
    b = np.asarray(b, np.float32).reshape(1, D)
    bres_v = np.asarray(bres, np.float32).reshape(1, D)
    edge_row = np.asarray(edge_row)
    edge_col = np.asarray(edge_col)
    edge_val = np.asarray(edge_val, np.float32)

    N = x.shape[0]
    Q = math.ceil(N / CH)
    nsh = math.ceil(N / N_CORES)
    n_blocks = math.ceil(nsh / P)
    nsh_pad = n_blocks * P
    n_groups = n_blocks * Q

    shards = []
    tpb = 1
    for c in range(N_CORES):
        lo = c * nsh
        hi = min(N, lo + nsh)
        m = (edge_row >= lo) & (edge_row < hi)
        r = (edge_row[m] - lo).astype(np.int64)
        ci = edge_col[m].astype(np.int64)
        v = edge_val[m]
        blk = r >> 7
        q = ci // CH
        counts = np.bincount(blk * Q + q, minlength=n_groups)
        tpb = max(tpb, int(math.ceil(counts.max() / P)))
        shards.append((r, ci, v, blk, q))

    G = Q * n_blocks * tpb
    IC = G * 8
    sbs = [list(range(s, min(s + SBW, n_blocks))) for s in range(0, n_blocks, SBW)]
    grp_tile0 = np.zeros((n_blocks, Q), np.int64)
    tcur = 0
    for sb in sbs:
        nb = len(sb)
        for q in range(Q):
            for j, k in enumerate(sb):
                grp_tile0[k, q] = tcur + j * tpb
            tcur += nb * tpb
    assert tcur == G

    iota_f = np.tile(np.arange(P, dtype=np.float32), (P, 1))

    in_maps = []
    for c in range(N_CORES):
        r, ci, v, blk, q = shards[c]
        gid = blk * Q + q
        order = np.argsort(gid, kind="stable")
        r, ci, v, blk, q, gid = (a[order] for a in (r, ci, v, blk, q, gid))
        starts = np.zeros(n_groups + 1, np.int64)
        np.cumsum(np.bincount(gid, minlength=n_groups), out=starts[1:])
        ranks = np.arange(len(r), dtype=np.int64) - starts[gid]
        slot = (grp_tile0[blk, q] + (ranks >> 7)) * P + (ranks & 127)

        idx16 = np.zeros(G * P, np.int16)
        d_flat = np.zeros(G * P, np.float32)
        v_flat = np.zeros(G * P, np.float32)
        idx16[slot] = (ci - q * CH).astype(np.int16)
        d_flat[slot] = (r & 127).astype(np.float32)
        v_flat[slot] = v
        idx_h = np.tile(np.ascontiguousarray(idx16.reshape(IC, 16).T), (8, 1))
        d_h = np.ascontiguousarray(d_flat.reshape(G, P).T)
        v_h = np.ascontiguousarray(v_flat.reshape(G, P).T)
        degv = np.zeros(nsh_pad, np.float32)
        degv[:nsh] += np.bincount(r, weights=v, minlength=nsh
                                  ).astype(np.float32)[:nsh]
        in_maps.append({
            "x": x, "W": W, "Wres": Wres, "bvec": b, "bres": bres_v,
            "iotaf": iota_f, "idx": idx_h, "darr": d_h, "varr": v_h,
            "deg": degv.reshape(1, nsh_pad),
        })
    meta = dict(N=N, nsh=nsh, n_blocks=n_blocks, nsh_pad=nsh_pad, tpb=tpb, Q=Q)
    return in_maps, meta


def kernel(x, W, b, Wres, bres, edge_val, edge_row, edge_col):
    in_maps, meta = _prep(x, W, b, Wres, bres, edge_val, edge_row, edge_col)
    nc = _build(np.asarray(x).shape[0], meta["n_blocks"], meta["tpb"])
    res = run_bass_kernel_spmd(nc, in_maps, core_ids=list(range(N_CORES)))
    N, nsh = meta["N"], meta["nsh"]
    out = np.empty((N, D), np.float32)
    for c in range(N_CORES):
        lo = c * nsh
        hi = min(N, lo + nsh)
        out[lo:hi] = res.results[c]["outT"].T[: hi - lo]
    return out



# revision 19
# speedup vs baseline: 1.5906x; 1.5906x over previous
"""GCNConv (graph message passing) on 8 Trainium2 NeuronCores — Bass/Tile.

out = a + (a @ Wres + bres),  a = relu(segment_sum(edge_val * (xW+b)[edge_col],
edge_row)),  computed via the identity  agg_lin = (A@x) @ W + deg x b  so the
sparse part runs on raw x, and the residual is fused as  out = a@(Wres+I)+bres.

Sharding: nodes (segment-sum destinations) are partitioned across the 8 cores
(12500 nodes each); x (host-cast to bf16) and the small dense weights are
replicated; each core processes exactly the edges whose destination lands in
its shard (host-side routing).

Per-core device algorithm (fully transposed, features on partitions):
  Phase 1, per superblock of SBW=8 destination blocks (128 dests each):
  for each of the Q=4 source chunks (int16 gather indices limit a chunk to
  <=32767 rows of x) one dma_gather pulls that chunk's edges' source rows —
  packed DENSELY in (block, slot) order, bf16, 256B per row — into an SBUF
  tile xg [128 slots, n_groups*128 feats].  Each 128-slot group g feeds one
  matmul per destination block it touches: psum[b] += xg[:, g].T @ S where
  S[slot, dest] = (iota==d)*v is built on the vector engine from per-slot
  dest-offset/value scalars (d=-1 masks slots of other blocks / padding, so
  block boundaries may fall mid-group without any padding).  Two PSUM banks
  [128 f, 4*128 d] accumulate across chunks, then flush to bf16 agg tiles.
  Gather indices and the per-instance d/v scalars stream per superblock so
  the first gathers are not serialized behind one big metadata load.
  Phase 2 (interleaved per superblock, slabs of 512 dests): psA = W.T @ agg
  + b x deg (rank-1), aT = relu(psA) in bf16; psB = (Wres+I).T @ aT + bres;
  outT[:, slab] = psB, stored transposed [128, 12544] per core; the host
  transposes + concatenates.
"""
import math

import numpy as np
from ml_dtypes import bfloat16

import concourse.tile as tile
from concourse import bacc, mybir
from concourse.bass_utils import run_bass_kernel_spmd

F32 = mybir.dt.float32
BF16 = mybir.dt.bfloat16
I16 = mybir.dt.int16
AL = mybir.AluOpType
D = 128
P = 128
N_CORES = 8
CH = 25000        # x chunk rows (int16 gather indices => <= 32767)
SBW = 4           # destination blocks per superblock
NQ = 4            # SWDGE queues
ALIGN = False     # 128-align each (block, chunk) segment (fewer matmuls,
                  # more gather padding) vs dense packing (opposite)


def _build(n_src, sched, repeat=1):
    n_blocks = sched["n_blocks"]
    nsh_pad = n_blocks * P
    Q = sched["Q"]
    XGW = sched["xgw"]          # fixed xg tile width (cols, feat-major)
    IC_MAX = sched["ic_max"]
    M_MAX = sched["m_max"]
    n_sb = len(sched["sbs"])

    nc = bacc.Bacc("TRN2", target_bir_lowering=False, debug=False,
                   num_swdge_queues=NQ)
    x = nc.dram_tensor("x", [n_src, D], BF16, kind="ExternalInput")
    W = nc.dram_tensor("W", [D, D], BF16, kind="ExternalInput")
    WresI = nc.dram_tensor("WresI", [D, D], BF16, kind="ExternalInput")
    bvec = nc.dram_tensor("bvec", [1, D], BF16, kind="ExternalInput")
    bres = nc.dram_tensor("bres", [1, D], BF16, kind="ExternalInput")
    iotaf = nc.dram_tensor("iotaf", [P, P], BF16, kind="ExternalInput")
    deg = nc.dram_tensor("deg", [1, nsh_pad], BF16, kind="ExternalInput")
    idx_t = [nc.dram_tensor(f"idx{k}", [P, sched["sbs"][k]["ic"]], I16,
                            kind="ExternalInput") for k in range(n_sb)]
    d_t = [nc.dram_tensor(f"darr{k}", [P, sched["sbs"][k]["m"]], F32,
                          kind="ExternalInput") for k in range(n_sb)]
    v_t = [nc.dram_tensor(f"varr{k}", [P, sched["sbs"][k]["m"]], F32,
                          kind="ExternalInput") for k in range(n_sb)]
    outT = nc.dram_tensor("outT", [D, nsh_pad], F32, kind="ExternalOutput")

    with tile.TileContext(nc) as tc:
        with tc.tile_pool(name="const", bufs=1) as cp:
            W_sb = cp.tile([D, D], BF16)
            nc.sync.dma_start(W_sb[:], W.ap())
            WresI_sb = cp.tile([D, D], BF16)
            nc.sync.dma_start(WresI_sb[:], WresI.ap())
            b_sb = cp.tile([1, D], BF16)
            nc.sync.dma_start(b_sb[:], bvec.ap())
            bres_sb = cp.tile([1, D], BF16)
            nc.sync.dma_start(bres_sb[:], bres.ap())
            deg_sb = cp.tile([1, nsh_pad], BF16)
            nc.sync.dma_start(deg_sb[:], deg.ap())
            iota_f = cp.tile([P, P], BF16)
            nc.sync.dma_start(iota_f[:], iotaf.ap())
            ones_row = cp.tile([1, 512], BF16)
            nc.vector.memset(ones_row[:], 1.0)

            for _rep in range(repeat):
                with (
                    tc.tile_pool(name="meta", bufs=4) as mp,
                    tc.tile_pool(name="xg", bufs=3 * Q) as xg_pool,
                    tc.tile_pool(name="s", bufs=32) as s_pool,
                    tc.tile_pool(name="agg", bufs=2) as agg_pool,
                    tc.tile_pool(name="ot", bufs=2) as o_pool,
                    tc.tile_pool(name="ps1", bufs=6 * ((SBW + 3) // 4),
                                 space="PSUM") as ps1,
                    tc.tile_pool(name="psA", bufs=1, space="PSUM") as psA_pool,
                    tc.tile_pool(name="psB", bufs=1, space="PSUM") as psB_pool,
                ):
                    for sbi, sb in enumerate(sched["sbs"]):
                        nb = sb["nb"]
                        col0 = sb["col0"]
                        ic, m_sb = sb["ic"], sb["m"]
                        idx_sb = mp.tile([P, IC_MAX], I16, tag="idx",
                                         name=f"idx{sbi}")
                        nc.sync.dma_start(idx_sb[:, :ic], idx_t[sbi].ap())
                        d_sb = mp.tile([P, M_MAX], F32, tag="d",
                                       name=f"d{sbi}")
                        nc.sync.dma_start(d_sb[:, :m_sb], d_t[sbi].ap())
                        v_sb = mp.tile([P, M_MAX], F32, tag="v",
                                       name=f"v{sbi}")
                        nc.sync.dma_start(v_sb[:, :m_sb], v_t[sbi].ap())
                        xgs = []
                        for q, nidx, ioff in sb["gathers"]:
                            xg = xg_pool.tile([P, XGW], BF16, tag="xg",
                                              name=f"xg{sbi}_{q}")
                            nc.gpsimd.dma_gather(
                                xg[:, :nidx].rearrange("p (g f) -> p g f", f=P),
                                x.ap()[q * CH: min(n_src, (q + 1) * CH), :],
                                idx_sb[:, ioff: ioff + nidx // 16],
                                nidx, nidx, D,
                                single_packet=(nidx <= 1024),
                                queue_num=q % NQ,
                            )
                            xgs.append(xg)
                        psbanks = [ps1.tile([P, 512], F32, tag="ps",
                                            name=f"ps{sbi}_{k}")
                                   for k in range((nb + 3) // 4)]
                        pss = [psbanks[j // 4][:, (j % 4) * P:(j % 4 + 1) * P]
                               for j in range(nb)]
                        for (qi, g, j, st, sp, m) in sb["instances"]:
                            S = s_pool.tile([P, P], BF16, name="S")
                            nc.vector.tensor_scalar(
                                S[:], iota_f[:],
                                d_sb[:, m:m + 1], v_sb[:, m:m + 1],
                                op0=AL.is_equal, op1=AL.mult,
                            )
                            nc.tensor.matmul(
                                out=pss[j][:],
                                lhsT=xgs[qi][:, g * P:(g + 1) * P],
                                rhs=S[:],
                                start=st, stop=sp,
                            )
                        agg_sb = agg_pool.tile([P, SBW * P], BF16, tag="agg",
                                               name="agg")
                        for j in range(nb):
                            nc.scalar.activation(
                                agg_sb[:, j * P:(j + 1) * P], pss[j][:],
                                mybir.ActivationFunctionType.Copy)
                        # dense head on this superblock's columns
                        for s0 in range(0, nb * P, 512):
                            w = min(512, nb * P - s0)
                            psA = psA_pool.tile([P, 512], F32, name="psA")
                            nc.tensor.matmul(out=psA[:, :w], lhsT=W_sb[:],
                                             rhs=agg_sb[:, s0:s0 + w],
                                             start=True, stop=False)
                            nc.tensor.matmul(
                                out=psA[:, :w], lhsT=b_sb[:1, :],
                                rhs=deg_sb[:1, col0 + s0: col0 + s0 + w],
                                start=False, stop=True)
                            a_t = agg_pool.tile([P, 512], BF16, tag="at",
                                                name="at")
                            nc.scalar.activation(
                                a_t[:, :w], psA[:, :w],
                                mybir.ActivationFunctionType.Relu)
                            psB = psB_pool.tile([P, 512], F32, name="psB")
                            nc.tensor.matmul(out=psB[:, :w], lhsT=WresI_sb[:],
                                             rhs=a_t[:, :w],
                                             start=True, stop=False)
                            nc.tensor.matmul(out=psB[:, :w],
                                             lhsT=bres_sb[:1, :],
                                             rhs=ones_row[:1, :w],
                                             start=False, stop=True)
                            o_t = o_pool.tile([P, 512], F32, name="ot")
                            nc.scalar.activation(
                                o_t[:, :w], psB[:, :w],
                                mybir.ActivationFunctionType.Copy)
                            nc.sync.dma_start(
                                outT.ap()[:, col0 + s0: col0 + s0 + w],
                                o_t[:, :w])

    nc.compile()
    return nc


def _prep(x, W, b, Wres, bres, edge_val, edge_row, edge_col):
    x = np.asarray(x, np.float32)
    n_src = x.shape[0]
    N = n_src
    x_bf = np.ascontiguousarray(x.astype(bfloat16))
    W_bf = np.ascontiguousarray(np.asarray(W, np.float32).astype(bfloat16))
    WresI = np.asarray(Wres, np.float32) + np.eye(D, dtype=np.float32)
    WresI_bf = np.ascontiguousarray(WresI.astype(bfloat16))
    b_bf = np.asarray(b, np.float32).reshape(1, D).astype(bfloat16)
    bres_bf = np.asarray(bres, np.float32).reshape(1, D).astype(bfloat16)
    edge_row = np.asarray(edge_row).astype(np.int64)
    edge_col = np.asarray(edge_col).astype(np.int64)
    edge_val = np.asarray(edge_val, np.float32)

    Q = math.ceil(n_src / CH)
    nsh = math.ceil(N / N_CORES)
    n_blocks = math.ceil(nsh / P)
    nsh_pad = n_blocks * P
    sb_list = []
    s = 0
    while s < n_blocks:
        rem = n_blocks - s
        if rem > SBW + 2:
            step = SBW
        elif rem > 4:                       # taper the tail: shorter drain
            step = rem - 4
        elif rem > 2:
            step = 2
        else:
            step = rem
        sb_list.append(list(range(s, s + step)))
        s += step
    n_sb = len(sb_list)
    NG = n_sb * Q                      # gather-group count
    blk2sb = np.empty(n_blocks, np.int64)
    blk2j = np.empty(n_blocks, np.int64)
    for si, sbl in enumerate(sb_list):
        for j, bb in enumerate(sbl):
            blk2sb[bb] = si
            blk2j[bb] = j

    # --- shard + sort edges per core, per-(sb,q) counts ---
    cores = []
    cnt = np.zeros((N_CORES, NG), np.int64)
    cnt2 = np.zeros((N_CORES, NG, SBW), np.int64)
    for c in range(N_CORES):
        lo = c * nsh
        m = (edge_row >= lo) & (edge_row < min(N, lo + nsh))
        r = edge_row[m] - lo
        ci = edge_col[m]
        v = edge_val[m]
        blk = r >> 7
        q = ci // CH
        sbid = blk2sb[blk]
        jloc = blk2j[blk]
        order = np.lexsort((jloc, q, sbid))
        r, ci, v, q, sbid, jloc = (a[order] for a in (r, ci, v, q, sbid, jloc))
        gid = sbid * Q + q
        cnt[c] = np.bincount(gid, minlength=NG)
        cnt2[c] = np.bincount(gid * SBW + jloc,
                              minlength=NG * SBW).reshape(NG, SBW)
        cores.append((r, ci, v, q, gid, jloc))

    if ALIGN:
        # each (gid, block) segment gets a fixed 128-aligned reservation:
        # no cross-block matmul instances, more gather padding
        wseg = -(-cnt2.max(axis=0) // P) * P                  # [NG, SBW]
        wseg[:, 0] = np.maximum(wseg[:, 0], P)
        seg_base = np.zeros((NG, SBW + 1), np.int64)
        np.cumsum(wseg, axis=1, out=seg_base[:, 1:])
        nidx_g = seg_base[:, -1]
        n_groups = nidx_g // P
    else:
        n_groups = np.maximum(1, -(-cnt.max(axis=0) // P))    # per gid
        nidx_g = n_groups * P
    slot_base = np.zeros(NG + 1, np.int64)
    np.cumsum(nidx_g, out=slot_base[1:])
    total_slots = int(slot_base[-1])
    G_MAX = int(n_groups.max())

    # --- instance list: per gid, which (group, block) pairs exist ---
    if ALIGN:
        Gs = seg_base[:, :-1] >> 7
        Ge = (seg_base[:, 1:] - 1) >> 7
        Ge = np.where(wseg > 0, Ge, Gs - 1)   # empty segment -> no instance
        Ge[:, 0] = np.maximum(Ge[:, 0], Gs[:, 0])
    else:
        s_cgj = np.zeros((N_CORES, NG, SBW + 1), np.int64)
        np.cumsum(cnt2, axis=2, out=s_cgj[:, :, 1:])
        starts = s_cgj[:, :, :-1]
        ends = s_cgj[:, :, 1:]
        has = cnt2 > 0
        gs = np.where(has, starts >> 7, np.iinfo(np.int64).max)
        ge = np.where(has, (ends - 1) >> 7, -1)
        Gs = gs.min(axis=0)            # [NG, SBW]
        Ge = ge.max(axis=0)
        # guarantee every block of every sb has at least one instance
        none = Ge < 0
        Gs[none] = 0
        Ge[none] = 0

    inst_keys = []
    for gidx in range(NG):
        for j in range(SBW):
            sbid = gidx // Q
            if j >= len(sb_list[sbid]):
                continue
            for g in range(Gs[gidx, j], Ge[gidx, j] + 1):
                inst_keys.append((gidx * G_MAX + g) * SBW + j)
    inst_keys = np.sort(np.array(inst_keys, np.int64))
    M = len(inst_keys)

    # decode instances; start/stop per PSUM bank (start zeroes a whole 2KB)
    first_of_bank = {}
    last_of_bank = {}
    inst_decode = []
    for m in range(M):
        k = int(inst_keys[m])
        j = k % SBW
        g = (k // SBW) % G_MAX
        gidx = k // (SBW * G_MAX)
        sbid, q = gidx // Q, gidx % Q
        inst_decode.append((sbid, q, g, j))
        bk = (sbid, j // 4)
        if bk not in first_of_bank:
            first_of_bank[bk] = m
        last_of_bank[bk] = m

    # per-sb schedules with sb-local idx offsets and instance ids
    sbs_sched = []
    m_lo = np.zeros(n_sb + 1, np.int64)       # instance-id range per sb
    for m in range(M):
        m_lo[inst_decode[m][0] + 1] = m + 1
    for sbid in range(n_sb):
        nb = len(sb_list[sbid])
        gathers = []
        ioff = 0
        for q in range(Q):
            gidx = sbid * Q + q
            gathers.append((q, int(nidx_g[gidx]), ioff))
            ioff += int(nidx_g[gidx]) // 16
        instances = []
        for m in range(int(m_lo[sbid]), int(m_lo[sbid + 1])):
            s_, q_, g_, j_ = inst_decode[m]
            assert s_ == sbid
            st = first_of_bank[(sbid, j_ // 4)] == m
            sp = last_of_bank[(sbid, j_ // 4)] == m
            instances.append((q_, g_, j_, st, sp, m - int(m_lo[sbid])))
        sbs_sched.append({"nb": nb, "col0": sb_list[sbid][0] * P,
                          "gathers": gathers, "instances": instances,
                          "ic": ioff, "m": len(instances)})

    sched = {"n_blocks": n_blocks, "Q": Q, "total_slots": total_slots,
             "M": M, "xgw": G_MAX * P, "sbs": sbs_sched,
             "ic_max": max(s["ic"] for s in sbs_sched),
             "m_max": max(s["m"] for s in sbs_sched)}

    # --- per-core tensors ---
    iota_np = np.tile(np.arange(P, dtype=np.float32),
                      (P, 1)).astype(bfloat16)
    in_maps = []
    for c in range(N_CORES):
        r, ci, v, q, gid, jloc = cores[c]
        if ALIGN:
            c2 = np.zeros(NG * SBW + 1, np.int64)
            np.cumsum(cnt2[c].reshape(-1), out=c2[1:])
            rank = np.arange(len(r), dtype=np.int64) - c2[gid * SBW + jloc]
            slot = slot_base[gid] + seg_base[gid, jloc] + rank
        else:
            gstart = np.zeros(NG + 1, np.int64)
            np.cumsum(cnt[c], out=gstart[1:])
            rank = np.arange(len(r), dtype=np.int64) - gstart[gid]
            slot = slot_base[gid] + rank

        idx16 = np.zeros(total_slots, np.int16)
        idx16[slot] = (ci - q * CH).astype(np.int16)

        ke = (gid * G_MAX + ((slot - slot_base[gid]) >> 7)) * SBW + jloc
        me = np.searchsorted(inst_keys, ke)
        assert (inst_keys[me] == ke).all()
        d_all = np.full((P, M), -1.0, np.float32)
        v_all = np.zeros((P, M), np.float32)
        d_all[slot & 127, me] = (r & 127).astype(np.float32)
        v_all[slot & 127, me] = v

        degv = np.zeros(nsh_pad, np.float32)
        lo = c * nsh
        hi = min(N, lo + nsh)
        degv[:hi - lo] = np.bincount(r, weights=v, minlength=hi - lo
                                     ).astype(np.float32)[:hi - lo]
        im = {
            "x": x_bf, "W": W_bf, "WresI": WresI_bf, "bvec": b_bf,
            "bres": bres_bf, "iotaf": iota_np,
            "deg": degv.astype(bfloat16).reshape(1, nsh_pad),
        }
        for sbid in range(n_sb):
            g0 = sbid * Q
            sl0, sl1 = int(slot_base[g0]), int(slot_base[g0 + Q])
            seg = idx16[sl0:sl1]
            im[f"idx{sbid}"] = np.tile(
                np.ascontiguousarray(seg.reshape(len(seg) // 16, 16).T),
                (8, 1))
            a0, a1 = int(m_lo[sbid]), int(m_lo[sbid + 1])
            im[f"darr{sbid}"] = np.ascontiguousarray(d_all[:, a0:a1])
            im[f"varr{sbid}"] = np.ascontiguousarray(v_all[:, a0:a1])
        in_maps.append(im)
    meta = dict(N=N, nsh=nsh, n_blocks=n_blocks, nsh_pad=nsh_pad, Q=Q)
    return in_maps, meta, sched


def kernel(x, W, b, Wres, bres, edge_val, edge_row, edge_col):
    in_maps, meta, sched = _prep(x, W, b, Wres, bres,
                                 edge_val, edge_row, edge_col)
    nc = _build(np.asarray(x).shape[0], sched)
    res = run_bass_kernel_spmd(nc, in_maps, core_ids=list(range(N_CORES)))
    N, nsh = meta["N"], meta["nsh"]
    out = np.empty((N, D), np.float32)
    for c in range(N_CORES):
        lo = c * nsh
        hi = min(N, lo + nsh)
        out[lo:hi] = res.results[c]["outT"].T[: hi - lo]
    return out
